# revision 40
# baseline (speedup 1.0000x reference)
"""Trainium2 Bass kernel v5 for nn_ContrastiveModel (ColBERT-style MaxSim).

score[b] = (sum_i max_j cos(a1[b,i], a2[b,j]) + sum_j max_i cos(...)) / (n1+n2)
with prefix validity masks (pos < sum(att_mask)).

Strategy (host marshals layout, device does all model math):
  - host: bf16 cast + [S,D]->[D,S] transpose, stack a1/a2 -> one DMA per batch
  - SBUF layout [128, 2, 6, 512]: partition p holds d = 6p+c, so main-matmul
    lhsT/rhs tiles come straight from DMA (no on-device marshal)
  - norms: squares (T1 on ACT, T2 on DVE) -> ones-matmul partition sums on PE;
    invalid tokens get +1e20 via a K=1 matmul of a precomputed row =>
    w = rsqrt(norm2) ~ 0 there (mask folded into the weights)
  - rsqrt on ACT via raw InstActivation (Rsqrt shares the act table with
    Square/Copy; tolerance here is 2e-2, far above its known inaccuracy)
  - post-matmul: ACT evacuates SIM with the per-partition w1 scale fused
    (Y = SIM*w1), DVE/GPS apply the broadcast w2 (Z = Y*W2B), one DVE reduce
    gives all four row-max columns, DVE fold chain + gpsimd
    partition_all_reduce(max) give the col-max, K=1 matmuls (deferred one
    batch to keep PE streaming) turn the colmax row into epilogue-summable
    columns
  - compiled with --enable-ldw-opt=true so LDWEIGHTS overlaps matmuls
"""

import os
import sys

sys.path.insert(0, "/opt/trn_rl_repo")

import numpy as np
import ml_dtypes
from contextlib import ExitStack

import concourse.bacc as bacc
import concourse.bass as bass
import concourse.tile as tile
from concourse import mybir
from concourse import bass_utils
from concourse._compat import with_exitstack

try:
    from concourse import bass_isa
except ImportError:
    import bass_isa  # type: ignore


def _axon_device_reset():
    import ctypes
    try:
        lib = ctypes.CDLL("/opt/axon/libaxon_pjrt.so")
        lib.axon_reset.restype = ctypes.c_int64
        rc = lib.axon_reset()
        if rc != 0:
            print("axon_reset rc:", rc)
    except Exception as e:
        print("axon_reset failed:", e)


_axon_device_reset()

N_CORES = 8
B_FULL, S, D = 64, 512, 768
BPC = B_FULL // N_CORES  # batches per core
NT = S // 128  # token blocks of the lhsT stationary (4)
NK = D // 128  # d blocks (6)
BIG = 1e20  # invalid-token norm2 bias => w = rsqrt(BIG) ~ 0

F32 = mybir.dt.float32
BF16 = mybir.dt.bfloat16
I32 = mybir.dt.int32
AX = mybir.AxisListType
ALU = mybir.AluOpType
ACTF = mybir.ActivationFunctionType
ROP = bass_isa.ReduceOp


def _act_raw(eng, out, in_, func):
    """activation() clone without the Rsqrt accuracy guard (tolerance 2e-2)."""
    b = eng.bass
    bias = b.const_aps.scalar_like(0.0, in_)
    ins_ = [
        eng.lower_ap(in_),
        eng.lower_ap(bias),
        mybir.ImmediateValue(dtype=mybir.dt.float32, value=1.0),
        mybir.ImmediateValue(dtype=mybir.dt.float32, value=0.0),
    ]
    outs_ = [eng.lower_ap(out)]
    return eng.add_instruction(
        mybir.InstActivation(
            name=b.get_next_instruction_name(), func=func, ins=ins_, outs=outs_
        )
    )


@with_exitstack
def _emit(ctx: ExitStack, tc: tile.TileContext, aps: dict):
    nc = tc.nc

    # per-batch view: partition p <- rows d = 6p..6p+5 of [D, S] (both tensors)
    ttr = aps["tt"].rearrange("b x (p c) j -> b p x c j", p=128)

    consts = ctx.enter_context(tc.tile_pool(name="consts", bufs=1))
    ttp = ctx.enter_context(tc.tile_pool(name="ttp", bufs=4))
    sq1p = ctx.enter_context(tc.tile_pool(name="sq1p", bufs=2))
    sq2p = ctx.enter_context(tc.tile_pool(name="sq2p", bufs=2))
    w2p = ctx.enter_context(tc.tile_pool(name="w2p", bufs=2))
    wrp = ctx.enter_context(tc.tile_pool(name="wrp", bufs=2))
    wcp = ctx.enter_context(tc.tile_pool(name="wcp", bufs=2))
    yp = ctx.enter_context(tc.tile_pool(name="yp", bufs=2))
    t2sp = ctx.enter_context(tc.tile_pool(name="t2sp", bufs=2))
    fp = ctx.enter_context(tc.tile_pool(name="fp", bufs=2))
    psS = ctx.enter_context(tc.tile_pool(name="psS", bufs=4, space="PSUM"))
    psN = ctx.enter_context(tc.tile_pool(name="psN", bufs=1, space="PSUM"))
    psB = ctx.enter_context(tc.tile_pool(name="psB", bufs=1, space="PSUM"))
    psW = ctx.enter_context(tc.tile_pool(name="psW", bufs=1, space="PSUM"))

    # ---- batch 0's T1 half first, tiny consts next, rest of inputs after ----
    TTa = ttp.tile([128, 2, NK, S], BF16, tag="tt")
    TTb = ttp.tile([128, 2, NK, S], BF16, tag="tt")
    TT01 = [TTa, TTb]
    nc.sync.dma_start(out=TT01[0][:, 0], in_=ttr[0, :, 0])
    IOTAR = consts.tile([1, S], F32, tag="iotar")
    nc.sync.dma_start(out=IOTAR[:], in_=aps["iotar"][:])
    ID8F = consts.tile([8, 8], F32, tag="id8f")
    nc.sync.dma_start(out=ID8F[:], in_=aps["id8f"][:])
    nc.sync.dma_start(out=TT01[0][:, 1], in_=ttr[0, :, 1])
    nc.sync.dma_start(out=TT01[1][:, 0], in_=ttr[1, :, 0])
    nc.sync.dma_start(out=TT01[1][:, 1], in_=ttr[1, :, 1])
    ONESB = consts.tile([128, 1], BF16, tag="onesb")
    nc.vector.memset(ONESB[:], 1.0)
    ONE1 = consts.tile([1, 1], BF16, tag="one1")
    nc.vector.memset(ONE1[:], 1.0)
    ONESR = consts.tile([1, 128], BF16, tag="onesr")
    nc.vector.memset(ONESR[:], 1.0)
    ONESF = consts.tile([128, 1], F32, tag="onesf")
    nc.vector.memset(ONESF[:], 1.0)

    # ---- masks -> n1,n2 -> rows on partition 0 ----
    M1i = consts.tile([BPC, S], I32, tag="m1i")
    nc.scalar.dma_start(out=M1i[:], in_=aps["m1"][:])
    M2i = consts.tile([BPC, S], I32, tag="m2i")
    nc.scalar.dma_start(out=M2i[:], in_=aps["m2"][:])
    M1f = consts.tile([BPC, S], F32, tag="m1f")
    nc.vector.tensor_copy(M1f[:], M1i[:])
    M2f = consts.tile([BPC, S], F32, tag="m2f")
    nc.vector.tensor_copy(M2f[:], M2i[:])
    n1 = consts.tile([BPC, 1], F32, tag="n1")
    nc.vector.tensor_reduce(out=n1[:], in_=M1f[:], axis=AX.X, op=ALU.add)
    n2 = consts.tile([BPC, 1], F32, tag="n2")
    nc.vector.tensor_reduce(out=n2[:], in_=M2f[:], axis=AX.X, op=ALU.add)
    ns = consts.tile([BPC, 1], F32, tag="ns")
    nc.vector.tensor_add(ns[:], n1[:], n2[:])

    # transpose n1/n2/ns to partition-0 rows via tiny f32 matmuls
    PR = psN.tile([1, 2 * S], F32, tag="pn")
    nc.tensor.matmul(out=PR[:1, 0:8], lhsT=n1[:], rhs=ID8F[:], start=True,
                     stop=True)
    nc.tensor.matmul(out=PR[:1, 8:16], lhsT=n2[:], rhs=ID8F[:], start=True,
                     stop=True)
    nc.tensor.matmul(out=PR[:1, 16:24], lhsT=ns[:], rhs=ID8F[:], start=True,
                     stop=True)
    n1T = consts.tile([1, BPC], F32, tag="n1t")
    nc.vector.tensor_copy(n1T[:], PR[:1, 0:8])
    n2T = consts.tile([1, BPC], F32, tag="n2t")
    nc.vector.tensor_copy(n2T[:], PR[:1, 8:16])
    nsT = consts.tile([1, BPC], F32, tag="nst")
    nc.vector.tensor_copy(nsT[:], PR[:1, 16:24])
    RNS = consts.tile([1, BPC], F32, tag="rns")
    nc.vector.reciprocal(RNS[:], nsT[:])

    # warmup matmuls on zeros keep HAM at full clock until batch 0's norm
    # matmuls are ready (emitted after the tiny mask-transpose matmuls)
    WD = consts.tile([128, S], BF16, tag="warmdat")
    nc.vector.memset(WD[:], 0.0)
    PSW0 = psS.tile([128, S], F32, tag="sim")
    for r in range(30):
        nc.tensor.matmul(out=PSW0[:], lhsT=WD[:, 0:128], rhs=WD[:],
                         start=(r == 0), stop=(r == 29))
    WD2 = consts.tile([1, 1], F32, tag="warmsink")
    nc.vector.tensor_copy(WD2[:], PSW0[0:1, 0:1])

    # INV rows (partition 0): (iota >= n) * BIG, one per batch per tensor
    INV1 = consts.tile([1, BPC, S], BF16, tag="inv1")
    INV2 = consts.tile([1, BPC, S], BF16, tag="inv2")
    for b in range(BPC):
        nc.vector.tensor_scalar(out=INV1[:, b, :], in0=IOTAR[:],
                                scalar1=n1T[:, b : b + 1], scalar2=BIG,
                                op0=ALU.is_ge, op1=ALU.mult)
        nc.vector.tensor_scalar(out=INV2[:, b, :], in0=IOTAR[:],
                                scalar1=n2T[:, b : b + 1], scalar2=BIG,
                                op0=ALU.is_ge, op1=ALU.mult)

    # result collectors
    RC = consts.tile([128, NT * BPC], F32, tag="rc")
    CS4 = consts.tile([128, NT * BPC], F32, tag="cs4")
    CMXA = consts.tile([1, BPC, S], BF16, tag="cmxa")

    state = {}

    def emit_cs4(bsrc, PW):
        """colmax row of batch bsrc -> per-partition columns via K=1 MMs."""
        for t in range(NT):
            nc.tensor.matmul(out=PW[:, NT + t : NT + t + 1],
                             lhsT=CMXA[:, bsrc, 128 * t : 128 * (t + 1)],
                             rhs=ONE1[:], start=True, stop=True)
        nc.vector.tensor_copy(CS4[:, NT * bsrc : NT * (bsrc + 1)],
                              PW[:, NT : 2 * NT])

    def emit_prepA(b):
        """DMA + squares + norm ones-matmuls + rsqrt for batch b."""
        if b < 2:
            TT = TT01[b]
        else:
            TT = ttp.tile([128, 2, NK, S], BF16, tag="tt")
            nc.sync.dma_start(out=TT[:, 0], in_=ttr[b, :, 0])
            nc.sync.dma_start(out=TT[:, 1], in_=ttr[b, :, 1])
        T1 = TT[:, 0]
        T2 = TT[:, 1]
        SQ1 = sq1p.tile([128, NK, S], BF16, tag="sq1")
        nc.scalar.activation(out=SQ1.rearrange("p c j -> p (c j)"),
                             in_=T1.rearrange("p c j -> p (c j)"),
                             func=ACTF.Square)
        SQ2 = sq2p.tile([128, NK, S], BF16, tag="sq2")
        nc.vector.tensor_tensor(out=SQ2.rearrange("p c j -> p (c j)"),
                                in0=T2.rearrange("p c j -> p (c j)"),
                                in1=T2.rearrange("p c j -> p (c j)"),
                                op=ALU.mult)
        PN = psN.tile([1, 2 * S], F32, tag="pn")
        for c in range(NK):
            nc.tensor.matmul(out=PN[:1, 0:S], lhsT=ONESB[:], rhs=SQ1[:, c, :],
                             start=(c == 0), stop=False)
        nc.tensor.matmul(out=PN[:1, 0:S], lhsT=ONE1[:], rhs=INV1[:, b, :],
                         start=False, stop=True)
        for c in range(NK):
            nc.tensor.matmul(out=PN[:1, S : 2 * S], lhsT=ONESB[:],
                             rhs=SQ2[:, c, :], start=(c == 0), stop=False)
        nc.tensor.matmul(out=PN[:1, S : 2 * S], lhsT=ONE1[:],
                         rhs=INV2[:, b, :], start=False, stop=True)
        WR = wrp.tile([1, 2 * S], BF16, tag="wr")
        _act_raw(nc.scalar, WR[:], PN[:1, :], ACTF.Rsqrt)
        state[b] = {"TT": TT, "WR": WR}

    def emit_prepB(b):
        """w1 columns, w2 broadcast, and the w2-scaled T2 for batch b."""
        st = state[b]
        WR = st["WR"]
        PW = psW.tile([128, 2 * NT], F32, tag="pw")
        for t in range(NT):
            nc.tensor.matmul(out=PW[:, t : t + 1],
                             lhsT=WR[:, 128 * t : 128 * (t + 1)],
                             rhs=ONE1[:], start=True, stop=True)
        PB = psB.tile([128, S], F32, tag="pb")
        nc.tensor.matmul(out=PB[:], lhsT=ONESR[:], rhs=WR[:, S : 2 * S],
                         start=True, stop=True)
        WC = wcp.tile([128, NT], F32, tag="wc")
        nc.vector.tensor_copy(WC[:], PW[:, :NT])
        W2B = w2p.tile([128, S], BF16, tag="w2b")
        nc.scalar.copy(W2B[:], PB[:])
        # T2s = T2 * w2[j] (k-planes share the broadcast W2B row via stride-0)
        T2 = st["TT"][:, 1]
        T2S = t2sp.tile([128, NK, S], BF16, tag="t2s")
        in0, in1 = bass.broadcast_tensor_aps(T2[:], W2B[:].rearrange(
            "p (c j) -> p c j", c=1))
        nc.vector.tensor_tensor(out=T2S[:], in0=in0, in1=in1, op=ALU.mult)
        st.update(PW=PW, WC=WC, T2S=T2S)

    def emit_main(b):
        """main matmuls + scale/reduce chain for batch b; colsum K=1
        matmuls of batch b-1 ride in this PE slot (CMX(b-1) is ready)."""
        st = state[b]
        T1 = st["TT"][:, 0]
        WC, T2S = st["WC"], st["T2S"]
        SIMs = []
        for t in range(NT):
            SIM = psS.tile([128, S], F32, tag="sim")
            for c in range(NK):
                nc.tensor.matmul(out=SIM[:],
                                 lhsT=T1[:, c, 128 * t : 128 * (t + 1)],
                                 rhs=T2S[:, c, :], start=(c == 0),
                                 stop=(c == NK - 1))
            SIMs.append(SIM)
        if b - 1 in state:
            del state[b - 1]

        # Y = SIM*w1 via ACT: with w2 already in T2S this IS the full sim
        Y = yp.tile([128, NT, S], BF16, tag="y")
        for t in range(NT):
            nc.scalar.activation(out=Y[:, t, :], in_=SIMs[t][:],
                                 func=ACTF.Copy, scale=WC[:, t : t + 1])
        # rowmax over j for all 4 i-blocks in one reduce
        nc.vector.tensor_reduce(out=RC[:, NT * b : NT * (b + 1)], in_=Y[:],
                                axis=AX.X, op=ALU.max)
        # fold over i-blocks, colmax across partitions
        F1 = fp.tile([128, S], BF16, tag="f1")
        nc.vector.tensor_tensor(out=F1[:], in0=Y[:, 0, :], in1=Y[:, 1, :],
                                op=ALU.max)
        F2 = fp.tile([128, S], BF16, tag="f2")
        nc.vector.tensor_tensor(out=F2[:], in0=F1[:], in1=Y[:, 2, :],
                                op=ALU.max)
        F3 = fp.tile([128, S], BF16, tag="f3")
        nc.vector.tensor_tensor(out=F3[:], in0=F2[:], in1=Y[:, 3, :],
                                op=ALU.max)
        CMX = fp.tile([128, S], BF16, tag="cmx")
        nc.gpsimd.partition_all_reduce(CMX[:], F3[:], channels=128,
                                       reduce_op=ROP.max)
        nc.vector.tensor_copy(CMXA[:, b, :], CMX[0:1, :])

    # software pipeline: prepA two batches ahead, prepB one ahead
    emit_prepA(0)
    emit_prepA(1)
    emit_prepB(0)
    for b in range(BPC):
        if b + 2 < BPC:
            emit_prepA(b + 2)
        if b + 1 < BPC:
            emit_prepB(b + 1)
        emit_main(b)

    # colmax rows -> per-partition columns via K=1 matmuls (all batches)
    for b0 in range(BPC):
        PWE = psW.tile([128, 2 * NT], F32, tag="pw")
        emit_cs4(b0, PWE)

    # ---- epilogue: scores = (rowsum + colsum) / (n1+n2) ----
    PE2 = psN.tile([1, 2 * S], F32, tag="pn")
    nc.tensor.matmul(out=PE2[:1, 0 : NT * BPC], lhsT=ONESF[:], rhs=RC[:],
                     start=True, stop=True)
    nc.tensor.matmul(out=PE2[:1, NT * BPC : 2 * NT * BPC], lhsT=ONESF[:],
                     rhs=CS4[:], start=True, stop=True)
    RS4 = consts.tile([1, BPC], F32, tag="rs4")
    nc.vector.tensor_reduce(out=RS4[:],
                            in_=PE2[:1, 0 : NT * BPC].rearrange(
                                "p (b t) -> p b t", t=NT),
                            axis=AX.X, op=ALU.add)
    CS8 = consts.tile([1, BPC], F32, tag="cs8")
    nc.vector.tensor_reduce(out=CS8[:],
                            in_=PE2[:1, NT * BPC : 2 * NT * BPC].rearrange(
                                "p (b t) -> p b t", t=NT),
                            axis=AX.X, op=ALU.add)
    TOT = consts.tile([1, BPC], F32, tag="tot")
    nc.vector.tensor_add(TOT[:], RS4[:], CS8[:])
    OUTT = consts.tile([1, BPC], F32, tag="outt")
    nc.vector.tensor_mul(OUTT[:], TOT[:], RNS[:])
    nc.sync.dma_start(out=aps["out"][:], in_=OUTT[:])
    if "dbg" in aps:
        nc.sync.dma_start(out=aps["dbg"][0:1], in_=RS4[:])
        nc.sync.dma_start(out=aps["dbg"][1:2], in_=CS8[:])


_CACHE = {}


def _patch_ldw_opt():
    """concourse compiles with --enable-ldw-opt=false; enable so LDWEIGHTS
    pipelines under the previous matmul's stream."""
    if getattr(bass_utils, "_ldw_patched", False):
        return
    orig = bass_utils.run_command

    def patched(argv, **kw):
        argv = [a.replace("--enable-ldw-opt=false", "--enable-ldw-opt=true")
                if isinstance(a, str) else a for a in argv]
        return orig(argv, **kw)

    bass_utils.run_command = patched
    bass_utils._ldw_patched = True


def _build():
    if "nc" in _CACHE:
        return _CACHE["nc"]
    if os.environ.get("KV2_LDW"):
        _patch_ldw_opt()
    nc = bacc.Bacc("TRN2", target_bir_lowering=False, debug=False,
                   num_devices=N_CORES)
    aps = {
        "tt": nc.dram_tensor("tt", [BPC, 2, D, S], BF16,
                             kind="ExternalInput").ap(),
        "m1": nc.dram_tensor("m1", [BPC, S], I32, kind="ExternalInput").ap(),
        "m2": nc.dram_tensor("m2", [BPC, S], I32, kind="ExternalInput").ap(),
        "iotar": nc.dram_tensor("iotar", [1, S], F32,
                                kind="ExternalInput").ap(),
        "id8f": nc.dram_tensor("id8f", [8, 8], F32, kind="ExternalInput").ap(),
        "out": nc.dram_tensor("out", [1, BPC], F32, kind="ExternalOutput").ap(),
    }
    if os.environ.get("KV2_DBG"):
        aps["dbg"] = nc.dram_tensor("dbg", [2, BPC], F32,
                                    kind="ExternalOutput").ap()
    with tile.TileContext(nc) as tc:
        _emit(tc, aps)
    nc.compile()
    _CACHE["nc"] = nc
    return nc


def _consts():
    return {
        "iotar": np.arange(S, dtype=np.float32).reshape(1, S),
        "id8f": np.eye(8, dtype=np.float32),
    }


def make_in_maps(article_1_emb, article_2_emb, article_1_att_mask,
                 article_2_att_mask):
    a1 = np.asarray(article_1_emb, dtype=np.float32)
    a2 = np.asarray(article_2_emb, dtype=np.float32)
    # bf16 cast + [B,S,D]->[B,D,S] transpose + stack: layout marshaling only
    t1 = np.ascontiguousarray(
        a1.astype(ml_dtypes.bfloat16).transpose(0, 2, 1))
    t2 = np.ascontiguousarray(
        a2.astype(ml_dtypes.bfloat16).transpose(0, 2, 1))
    tt = np.stack([t1, t2], axis=1)  # [B, 2, D, S]
    m1 = np.ascontiguousarray(np.asarray(article_1_att_mask, dtype=np.int32))
    m2 = np.ascontiguousarray(np.asarray(article_2_att_mask, dtype=np.int32))
    cst = _consts()
    in_maps = []
    for c in range(N_CORES):
        sl = slice(c * BPC, (c + 1) * BPC)
        in_maps.append({"tt": tt[sl], "m1": m1[sl], "m2": m2[sl], **cst})
    return in_maps


def _ensure_profile_hook():
    import types

    if "antenv.axon_hooks" in sys.modules:
        return
    mod = types.ModuleType("antenv.axon_hooks")
    mod._hook = None
    mod.set_axon_ntff_profile_hook = lambda h: setattr(mod, "_hook", h)
    mod.get_axon_ntff_profile_hook = lambda: mod._hook
    sys.modules["antenv.axon_hooks"] = mod
    try:
        from trn_agent_boot.trn_boot import _ntff_profile_via_ctypes
        mod._hook = _ntff_profile_via_ctypes("/opt/axon/libaxon_pjrt.so")
    except Exception as e:
        print("ntff hook setup failed:", e)


def kernel(article_1_emb, article_2_emb, article_1_att_mask,
           article_2_att_mask, _trace=False, _trace_kwargs=None):
    if _trace:
        _ensure_profile_hook()
    nc = _build()
    in_maps = make_in_maps(article_1_emb, article_2_emb, article_1_att_mask,
                           article_2_att_mask)
    res = bass_utils.run_bass_kernel_spmd(
        nc, in_maps, core_ids=list(range(N_CORES)), trace=_trace,
        **(_trace_kwargs or {}))
    out = np.concatenate([np.asarray(res.results[c]["out"]).reshape(BPC)
                          for c in range(N_CORES)])
    if _trace:
        return out.astype(np.float32), res
    return out.astype(np.float32)


if __name__ == "__main__":
    rng = np.random.default_rng(0)
    a1 = rng.standard_normal((BPC, S, D), dtype=np.float32)
    a2 = rng.standard_normal((BPC, S, D), dtype=np.float32)
    m1 = rng.integers(0, 2, size=(BPC, S)).astype(np.int32)
    m2 = rng.integers(0, 2, size=(BPC, S)).astype(np.int32)

    nc = _build()
    print("compiled ok", flush=True)

    t1 = np.ascontiguousarray(a1.astype(ml_dtypes.bfloat16).transpose(0, 2, 1))
    t2 = np.ascontiguousarray(a2.astype(ml_dtypes.bfloat16).transpose(0, 2, 1))
    tt = np.stack([t1, t2], axis=1)

    from concourse.bass_interp import CoreSim
    sim = CoreSim(nc)
    cst = _consts()
    for k, v in (("tt", tt), ("m1", m1), ("m2", m2), *cst.items()):
        sim.tensor(k)[:] = v
    sim.simulate()
    got = np.asarray(sim.tensor("out")).reshape(BPC)
    if os.environ.get("KV2_DBG"):
        dbg = np.asarray(sim.tensor("dbg"))
        print("rowsum:", dbg[0])
        print("colsum:", dbg[1])

    n1 = m1.sum(-1); n2 = m2.sum(-1)
    pos = np.arange(S)
    w1 = (pos[None, :] < n1[:, None]) / np.linalg.norm(a1, axis=-1)
    w2 = (pos[None, :] < n2[:, None]) / np.linalg.norm(a2, axis=-1)
    M = np.einsum("bid,bjd->bij", a1 * w1[..., None], a2 * w2[..., None])
    want = (M.max(2).sum(-1) + M.max(1).sum(-1)) / (n1 + n2)
    print("sim:", got)
    print("ref:", want)
    print("rel err:", np.abs(got - want).max() / np.abs(want).max())
